# revision 1
# baseline (speedup 1.0000x reference)
"""Trainium2 Bass kernel for nn_ContinuousConvolutionBlock (gnn_message_passing).

Strategy (per sharding hint: partition points across 8 cores; each core owns its
queries' scatter-reduce and tap-GEMM; filter + dense weights replicated):

Host side (index plumbing / input marshalling only — zero FLOPs):
  - qry_idx is sorted; queries are grouped into 8-query blocks, blocks paired
    into 128-edge-slot "chunks" (two-pointer bin packing, ~3% padding).
  - Consecutive block ranges are assigned to the 8 cores; per-core per-slot
    payload arrays (pos[src], pos[qry], feats[src], local query id) are
    marshalled on host and DMA'd in dense [128 x NCH x k] layout.

Device side (all FLOP-bearing compute):
  - Geometry: ball->cube volume-preserving map + trilinear corner weights
    (DVE arithmetic + ACT sqrt/arctan/sign/abs), producing per-slot 4-wide
    one-hot weight vectors w4x/w4y/w4z (separable trilinear factorization).
  - Scatter-reduce as factored matmul per chunk: with R[slot,(ax,c)] =
    w4x (x) feats and L[slot,(q,az,ay)] = Qoh (x) w4z (x) w4y, PE computes
    A^T[(ax,c),(q,az,ay)] = R^T @ L, accumulating the per-query tap grid
    A[q, az,ay,ax, c] directly in transposed layout (PSUM).
  - Tap-GEMM: for each (az,ay) tap-pair t, out^T += G_t^T @ A^T-slices,
    accumulated over 16 t in PSUM. G is the filter regrouped on host
    (pure relayout, replicated to all cores).
  - Dense branch: out_dense^T = dense_w^T @ feats^T + b on PE.
  Outputs are produced transposed ([64, nq]); host transposes/reorders back.
"""
import sys
import os
sys.path.insert(0, '/opt/trn_rl_repo')
import numpy as np

N = 30000
CIN = 32
COUT = 64
KS = 4
EXTENT = 0.08
NCORES = 8
NBLK = N // 8  # 3750 eight-query blocks

_COMPILED = {}


# ----------------------------------------------------------------------------
# Host planning
# ----------------------------------------------------------------------------
def _plan(qry_idx):
    deg = np.bincount(qry_idx, minlength=N)
    bsz = deg.reshape(NBLK, 8).sum(1)
    bstart = np.concatenate([[0], np.cumsum(bsz)]).astype(np.int64)
    per = [NBLK // NCORES + (1 if c < NBLK % NCORES else 0) for c in range(NCORES)]
    b0 = np.concatenate([[0], np.cumsum(per)]).astype(np.int64)
    plans = []
    for c in range(NCORES):
        blocks = list(range(b0[c], b0[c + 1]))
        asc = sorted(blocks, key=lambda b: bsz[b])
        chunks = []
        lo, hi = 0, len(asc) - 1
        while lo <= hi:
            if lo == hi:
                chunks.append((asc[hi], None)); break
            if bsz[asc[hi]] + bsz[asc[lo]] <= 128:
                chunks.append((asc[hi], asc[lo])); hi -= 1; lo += 1
            else:
                chunks.append((asc[hi], None)); hi -= 1
        plans.append(dict(blocks=blocks, chunks=chunks, q0=int(8 * b0[c]),
                          nq=int(8 * (b0[c + 1] - b0[c]))))
    return plans, bstart, bsz


def _pack_core(plan_c, bstart, pos, feats, qry_idx, src_idx, NCHP):
    """Build per-slot payload arrays in [128, NCHP, k] layout."""
    possrc = np.zeros((128, NCHP, 4), np.float32)
    posqry = np.zeros((128, NCHP, 4), np.float32)
    fsrc = np.zeros((128, NCHP, CIN), np.float32)
    qlocf = np.full((128, NCHP), -1.0, np.float32)
    for ci, (bA, bB) in enumerate(plan_c['chunks']):
        s = 0
        for half, b in enumerate((bA, bB)):
            if b is None:
                continue
            e0, e1 = int(bstart[b]), int(bstart[b + 1])
            n = e1 - e0
            sl = slice(s, s + n)
            possrc[sl, ci, 0:3] = pos[src_idx[e0:e1]]
            posqry[sl, ci, 0:3] = pos[qry_idx[e0:e1]]
            fsrc[sl, ci, :] = feats[src_idx[e0:e1]]
            qlocf[sl, ci] = (qry_idx[e0:e1] - 8 * b) + 8 * half
            s += n
    return possrc, posqry, fsrc, qlocf


# ----------------------------------------------------------------------------
# Device kernel
# ----------------------------------------------------------------------------
def _build_bass(NCHP, NQ):
    import concourse.bass as bass
    import concourse.tile as tile
    from concourse import bacc, mybir
    from concourse.bass import AP

    f32 = mybir.dt.float32
    f32r = mybir.dt.float32r
    i32 = mybir.dt.int32
    ALU = mybir.AluOpType
    ACT = mybir.ActivationFunctionType
    EPS = 1e-12
    F4PI = float(4.0 / np.pi)

    nc = bacc.Bacc("TRN2", target_bir_lowering=False, debug=False)

    possrc = nc.dram_tensor("possrc", (128, NCHP, 4), f32, kind="ExternalInput")
    posqry = nc.dram_tensor("posqry", (128, NCHP, 4), f32, kind="ExternalInput")
    fsrc = nc.dram_tensor("fsrc", (128, NCHP, CIN), f32, kind="ExternalInput")
    qlocf = nc.dram_tensor("qlocf", (128, NCHP), f32, kind="ExternalInput")
    g2 = nc.dram_tensor("g2", (128, 16 * 64), f32, kind="ExternalInput")
    featsT = nc.dram_tensor("featsT", (CIN, NQ), f32, kind="ExternalInput")
    denw = nc.dram_tensor("denw", (CIN, COUT), f32, kind="ExternalInput")
    denb = nc.dram_tensor("denb", (COUT, 1), f32, kind="ExternalInput")

    outconvT = nc.dram_tensor("outconvT", (COUT, NQ), f32, kind="ExternalOutput")
    outdenseT = nc.dram_tensor("outdenseT", (COUT, NQ), f32, kind="ExternalOutput")

    W = NCHP            # geometry tile width (all chunks at once)
    NGRP = NCHP // 16   # tap-GEMM groups

    with tile.TileContext(nc) as tc:
        with tc.tile_pool(name="inp", bufs=1) as inp, \
             tc.tile_pool(name="geo", bufs=1) as geo, \
             tc.tile_pool(name="tmp", bufs=1) as tmp, \
             tc.tile_pool(name="lr", bufs=10) as lrp, \
             tc.tile_pool(name="at", bufs=3) as atp, \
             tc.tile_pool(name="outp", bufs=4) as outp, \
             tc.tile_pool(name="ps1", bufs=4, space="PSUM") as ps1, \
             tc.tile_pool(name="ps2", bufs=2, space="PSUM") as ps2:

            # ---------------- input DMAs ----------------
            t_ps = inp.tile([128, W, 4], f32)
            t_pq = inp.tile([128, W, 4], f32)
            t_f = inp.tile([128, W, CIN], f32)
            t_ql = inp.tile([128, W], f32)
            t_g2 = inp.tile([128, 16 * 64], f32)
            t_ftT = inp.tile([CIN, NQ], f32)
            t_dw = inp.tile([CIN, COUT], f32)
            t_db = inp.tile([COUT, 1], f32)
            nc.sync.dma_start(t_ps[:], possrc[:])
            nc.sync.dma_start(t_pq[:], posqry[:])
            nc.sync.dma_start(t_f[:], fsrc[:])
            nc.sync.dma_start(t_ql[:], qlocf[:])
            nc.sync.dma_start(t_g2[:], g2[:])
            nc.sync.dma_start(t_ftT[:], featsT[:])
            nc.sync.dma_start(t_dw[:], denw[:])
            nc.sync.dma_start(t_db[:], denb[:])

            # round filter to f32r once
            t_g2r = inp.tile([128, 16 * 64], f32r)
            nc.vector.tensor_copy(t_g2r[:], t_g2[:])

            # iota constants
            io4i = tmp.tile([128, 4], i32)
            nc.gpsimd.iota(io4i[:], pattern=[[1, 4]], base=0, channel_multiplier=0)
            io4 = geo.tile([128, 4], f32)
            nc.vector.tensor_copy(io4[:], io4i[:])
            io16i = tmp.tile([128, 16], i32)
            nc.gpsimd.iota(io16i[:], pattern=[[1, 16]], base=0, channel_multiplier=0)
            io16 = geo.tile([128, 16], f32)
            nc.vector.tensor_copy(io16[:], io16i[:])

            # ---------------- geometry ----------------
            _tn = [0]
            _free_tags = []
            _tag_of = {}

            _seq = [0]

            def T(shape=(128, W), dt_=f32):
                if _free_tags:
                    tg = _free_tags.pop()
                else:
                    _tn[0] += 1
                    tg = f"t{_tn[0]}"
                _seq[0] += 1
                t = tmp.tile(list(shape), dt_, name=f"{tg}_u{_seq[0]}", tag=tg)
                _tag_of[id(t)] = tg
                return t

            def F(*ts):
                for t in ts:
                    _free_tags.append(_tag_of.pop(id(t)))

            TT = nc.vector.tensor_tensor
            TS = nc.vector.tensor_scalar
            STT = nc.vector.scalar_tensor_tensor

            # r = (ps - pq) * (2/EXTENT), per coord [128, W, 3]
            r = T((128, W, 3))
            TT(out=r[:], in0=t_ps[:, :, 0:3], in1=t_pq[:, :, 0:3], op=ALU.subtract)
            rs = T((128, W, 3))
            TS(rs[:], r[:], float(2.0 / EXTENT), None, op0=ALU.mult)
            F(r)
            x, y, z = rs[:, :, 0], rs[:, :, 1], rs[:, :, 2]

            sq3 = T((128, W, 3))
            TT(out=sq3[:], in0=rs[:], in1=rs[:], op=ALU.mult)
            x2, y2, z2 = sq3[:, :, 0], sq3[:, :, 1], sq3[:, :, 2]
            xy2 = T()
            TT(out=xy2[:], in0=x2, in1=y2, op=ALU.add)
            sq = T()
            TT(out=sq[:], in0=xy2[:], in1=z2, op=ALU.add)

            norm = T()
            nc.scalar.activation(norm[:], sq[:], ACT.Sqrt)
            nxy = T()
            nc.scalar.activation(nxy[:], xy2[:], ACT.Sqrt)

            p125 = T()
            TS(p125[:], z2, 1.25, None, op0=ALU.mult)
            pole = T()
            TT(out=pole[:], in0=p125[:], in1=xy2[:], op=ALU.is_gt)
            F(sq3, xy2, p125)

            azn = T()
            nc.scalar.activation(azn[:], z, ACT.Abs)
            den1 = T()
            STT(out=den1[:], in0=azn[:], scalar=EPS, in1=norm[:], op0=ALU.add, op1=ALU.add)
            rd1 = T()
            nc.vector.reciprocal(rd1[:], den1[:])
            t1s = T()
            STT(out=t1s[:], in0=norm[:], scalar=3.0, in1=rd1[:], op0=ALU.mult, op1=ALU.mult)
            s1 = T()
            nc.scalar.activation(s1[:], t1s[:], ACT.Sqrt)
            F(azn, den1, rd1, t1s)

            den2 = T()
            TS(den2[:], nxy[:], EPS, None, op0=ALU.add)
            rd2 = T()
            nc.vector.reciprocal(rd2[:], den2[:])
            s2 = T()
            TT(out=s2[:], in0=norm[:], in1=rd2[:], op=ALU.mult)
            F(nxy, den2, rd2)

            d12 = T()
            TT(out=d12[:], in0=s1[:], in1=s2[:], op=ALU.subtract)
            pw = T()
            TT(out=pw[:], in0=pole[:], in1=d12[:], op=ALU.mult)
            wq = T()
            TT(out=wq[:], in0=s2[:], in1=pw[:], op=ALU.add)
            F(s1, s2, d12, pw)

            xc = T()
            TT(out=xc[:], in0=x, in1=wq[:], op=ALU.mult)
            yc = T()
            TT(out=yc[:], in0=y, in1=wq[:], op=ALU.mult)
            F(wq)

            sgz = T()
            nc.scalar.activation(sgz[:], z, ACT.Sign)
            zcp = T()
            TT(out=zcp[:], in0=sgz[:], in1=norm[:], op=ALU.mult)
            zce = T()
            TS(zce[:], z, 1.5, None, op0=ALU.mult)
            dz = T()
            TT(out=dz[:], in0=zcp[:], in1=zce[:], op=ALU.subtract)
            pz = T()
            TT(out=pz[:], in0=pole[:], in1=dz[:], op=ALU.mult)
            zc = T()
            TT(out=zc[:], in0=zce[:], in1=pz[:], op=ALU.add)
            F(sgz, zcp, zce, dz, pz, pole, norm, rs)

            zero1 = T()
            TS(zero1[:], sq[:], EPS, None, op0=ALU.is_lt)
            onem1 = T()
            TS(onem1[:], zero1[:], -1.0, 1.0, op0=ALU.mult, op1=ALU.add)
            for t_ in (xc, yc, zc):
                TT(out=t_[:], in0=t_[:], in1=onem1[:], op=ALU.mult)
            F(sq, zero1, onem1)

            # cylinder -> cube
            xc2 = T()
            TT(out=xc2[:], in0=xc[:], in1=xc[:], op=ALU.mult)
            yc2 = T()
            TT(out=yc2[:], in0=yc[:], in1=yc[:], op=ALU.mult)
            sqxy = T()
            TT(out=sqxy[:], in0=xc2[:], in1=yc2[:], op=ALU.add)
            nrm = T()
            nc.scalar.activation(nrm[:], sqxy[:], ACT.Sqrt)
            F(xc2, yc2)

            axc = T()
            nc.scalar.activation(axc[:], xc[:], ACT.Abs)
            ayc = T()
            nc.scalar.activation(ayc[:], yc[:], ACT.Abs)
            abr = T()
            TT(out=abr[:], in0=ayc[:], in1=axc[:], op=ALU.is_le)

            mx = T()
            TS(mx[:], axc[:], EPS, None, op0=ALU.is_lt)
            sfx = T()
            TT(out=sfx[:], in0=xc[:], in1=mx[:], op=ALU.add)
            my = T()
            TS(my[:], ayc[:], EPS, None, op0=ALU.is_lt)
            sfy = T()
            TT(out=sfy[:], in0=yc[:], in1=my[:], op=ALU.add)
            F(axc, ayc, mx, my)

            rsx = T()
            nc.vector.reciprocal(rsx[:], sfx[:])
            rsy = T()
            nc.vector.reciprocal(rsy[:], sfy[:])
            ratx = T()
            TT(out=ratx[:], in0=xc[:], in1=rsy[:], op=ALU.mult)
            raty = T()
            TT(out=raty[:], in0=yc[:], in1=rsx[:], op=ALU.mult)
            at1 = T()
            nc.scalar.activation(at1[:], ratx[:], ACT.Arctan)
            at2 = T()
            nc.scalar.activation(at2[:], raty[:], ACT.Arctan)
            F(sfx, sfy, rsx, rsy, ratx, raty)

            sgx = T()
            nc.scalar.activation(sgx[:], xc[:], ACT.Sign)
            sgy = T()
            nc.scalar.activation(sgy[:], yc[:], ACT.Sign)
            tmpa = T()
            TT(out=tmpa[:], in0=sgx[:], in1=nrm[:], op=ALU.mult)
            tmpb = T()
            TT(out=tmpb[:], in0=sgy[:], in1=nrm[:], op=ALU.mult)
            F(sgx, sgy, nrm, xc, yc)

            # xo = where(a, tmpa, tmpb * F4PI * at1)
            xoe = T()
            STT(out=xoe[:], in0=at1[:], scalar=F4PI, in1=tmpb[:], op0=ALU.mult, op1=ALU.mult)
            dxo = T()
            TT(out=dxo[:], in0=tmpa[:], in1=xoe[:], op=ALU.subtract)
            adx = T()
            TT(out=adx[:], in0=abr[:], in1=dxo[:], op=ALU.mult)
            xo = T()
            TT(out=xo[:], in0=xoe[:], in1=adx[:], op=ALU.add)
            # yo = where(a, tmpa * F4PI * at2, tmpb)
            yoe = T()
            STT(out=yoe[:], in0=at2[:], scalar=F4PI, in1=tmpa[:], op0=ALU.mult, op1=ALU.mult)
            dyo = T()
            TT(out=dyo[:], in0=yoe[:], in1=tmpb[:], op=ALU.subtract)
            ady = T()
            TT(out=ady[:], in0=abr[:], in1=dyo[:], op=ALU.mult)
            yo = T()
            TT(out=yo[:], in0=tmpb[:], in1=ady[:], op=ALU.add)
            F(at1, at2, xoe, dxo, adx, yoe, dyo, ady, tmpa, tmpb, abr)

            zero2 = T()
            TS(zero2[:], sqxy[:], EPS, None, op0=ALU.is_lt)
            onem2 = T()
            TS(onem2[:], zero2[:], -1.0, 1.0, op0=ALU.mult, op1=ALU.add)
            TT(out=xo[:], in0=xo[:], in1=onem2[:], op=ALU.mult)
            TT(out=yo[:], in0=yo[:], in1=onem2[:], op=ALU.mult)
            F(sqxy, zero2, onem2)

            # ---------------- corner weights w4 ----------------
            def corners_w4(m_ap, w4_t):
                g = T()
                TS(g[:], m_ap, 1.5, 1.5, op0=ALU.mult, op1=ALU.add)
                gc = T()
                TS(gc[:], g[:], 0.0, None, op0=ALU.max)
                g0i = T(dt_=i32)
                TS(g0i[:], gc[:], 0.5, None, op0=ALU.subtract)  # cast rint => floor
                g0 = T()
                nc.vector.tensor_copy(g0[:], g0i[:])
                fr = T()
                TT(out=fr[:], in0=gc[:], in1=g0[:], op=ALU.subtract)
                i0 = T()
                TS(i0[:], g0[:], 3.0, None, op0=ALU.min)
                i1 = T()
                TS(i1[:], g0[:], 1.0, 3.0, op0=ALU.add, op1=ALU.min)
                # e0/e1 one-hots [128, W, 4]
                e0 = T((128, W, 4))
                TT(out=e0[:],
                   in0=AP(io4.tensor, io4[:].offset, [io4[:].ap[0], [0, W], [1, 4]]),
                   in1=AP(i0.tensor, i0[:].offset, [i0[:].ap[0], [1, W], [0, 4]]),
                   op=ALU.is_equal)
                e1 = T((128, W, 4))
                TT(out=e1[:],
                   in0=AP(io4.tensor, io4[:].offset, [io4[:].ap[0], [0, W], [1, 4]]),
                   in1=AP(i1.tensor, i1[:].offset, [i1[:].ap[0], [1, W], [0, 4]]),
                   op=ALU.is_equal)
                onemf = T()
                TS(onemf[:], fr[:], -1.0, 1.0, op0=ALU.mult, op1=ALU.add)
                TT(out=e0[:], in0=e0[:],
                   in1=AP(onemf.tensor, onemf[:].offset, [onemf[:].ap[0], [1, W], [0, 4]]),
                   op=ALU.mult)
                TT(out=e1[:], in0=e1[:],
                   in1=AP(fr.tensor, fr[:].offset, [fr[:].ap[0], [1, W], [0, 4]]),
                   op=ALU.mult)
                TT(out=w4_t[:], in0=e0[:], in1=e1[:], op=ALU.add)
                F(g, gc, g0i, g0, fr, i0, i1, e0, e1, onemf)

            w4x = geo.tile([128, W, 4], f32)
            w4y = geo.tile([128, W, 4], f32)
            w4z = geo.tile([128, W, 4], f32)
            corners_w4(xo[:], w4x)
            corners_w4(yo[:], w4y)
            corners_w4(zc[:], w4z)
            F(xo, yo, zc)

            # Qoh16 [128, W, 16], ZY [128, W, 16]
            qoh = geo.tile([128, W, 16], f32)
            TT(out=qoh[:],
               in0=AP(t_ql.tensor, t_ql[:].offset, [t_ql[:].ap[0], [1, W], [0, 16]]),
               in1=AP(io16.tensor, io16[:].offset, [io16[:].ap[0], [0, W], [1, 16]]),
               op=ALU.is_equal)
            zy = geo.tile([128, W, 16], f32)
            TT(out=zy[:],
               in0=AP(w4z.tensor, w4z[:].offset,
                      [w4z[:].ap[0], [4, W], [1, 4], [0, 4]]),
               in1=AP(w4y.tensor, w4y[:].offset,
                      [w4y[:].ap[0], [4, W], [0, 4], [1, 4]]),
               op=ALU.mult)

            # ---------------- stage-1 + tap-GEMM ----------------
            for g in range(NGRP):
                at_st = atp.tile([128, 16 * 256], f32r, tag="at")
                for cl in range(0, 16, 2):
                    ps_t = ps1.tile([128, 512], f32, space="PSUM", tag="s1")
                    for par in range(2):
                        ci = g * 16 + cl + par
                        # R [128, (ax, c)]
                        R = lrp.tile([128, 128], f32r, tag="R")
                        wx = w4x[:, ci, :]
                        ff = t_f[:, ci, :]
                        TT(out=AP(R.tensor, R[:].offset, [R[:].ap[0], [32, 4], [1, 32]]),
                           in0=AP(wx.tensor, wx.offset, [wx.ap[0], [1, 4], [0, 32]]),
                           in1=AP(ff.tensor, ff.offset, [ff.ap[0], [0, 4], [1, 32]]),
                           op=ALU.mult)
                        # L [128, (half, q, t)]
                        L = lrp.tile([128, 256], f32r, tag="L")
                        qq = qoh[:, ci, :]
                        zz = zy[:, ci, :]
                        TT(out=AP(L.tensor, L[:].offset,
                                  [L[:].ap[0], [128, 2], [16, 8], [1, 16]]),
                           in0=AP(qq.tensor, qq.offset,
                                  [qq.ap[0], [8, 2], [1, 8], [0, 16]]),
                           in1=AP(zz.tensor, zz.offset,
                                  [zz.ap[0], [0, 2], [0, 8], [1, 16]]),
                           op=ALU.mult)
                        nc.tensor.matmul(
                            out=ps_t[:, par * 256:(par + 1) * 256],
                            lhsT=R[:], rhs=L[:], start=True, stop=True)
                    # copy 2 chunks at once, alternating DVE/ACT
                    dst = at_st[:, cl * 256:(cl + 2) * 256]
                    if (cl // 2) % 2 == 0:
                        nc.vector.tensor_copy(dst, ps_t[:])
                    else:
                        nc.scalar.copy(dst, ps_t[:])
                # tap-GEMM for this group
                po = ps2.tile([COUT, 256], f32, space="PSUM", tag="tap")
                for t in range(16):
                    rhs = AP(at_st.tensor, at_st[:].offset + t,
                             [at_st[:].ap[0], [256, 16], [128, 2], [16, 8]])
                    nc.tensor.matmul(
                        out=po[:],
                        lhsT=t_g2r[:, t * 64:(t + 1) * 64],
                        rhs=rhs,
                        start=(t == 0), stop=(t == 15))
                ost = outp.tile([COUT, 256], f32, tag="ocst")
                nc.vector.tensor_copy(ost[:], po[:])
                nc.sync.dma_start(outconvT[:, g * 256:(g + 1) * 256], ost[:])

            # ---------------- dense branch (plain fp32 matmul) ----------------
            NSEG = (NQ + 511) // 512
            for s in range(NSEG):
                j0 = s * 512
                j1 = min(NQ, j0 + 512)
                pd = ps2.tile([COUT, 512], f32, space="PSUM", tag="den")
                nc.tensor.matmul(
                    out=pd[:, 0:j1 - j0],
                    lhsT=t_dw[:],
                    rhs=t_ftT[:, j0:j1],
                    start=True, stop=True)
                db = t_db[:, 0:1]
                odt = outp.tile([COUT, 512], f32, tag="odst")
                TT(out=odt[:, 0:j1 - j0], in0=pd[:, 0:j1 - j0],
                   in1=AP(db.tensor, db.offset, [db.ap[0], [0, j1 - j0]]),
                   op=ALU.add)
                nc.sync.dma_start(outdenseT[:, j0:j1], odt[:, 0:j1 - j0])

    nc.compile()
    return nc


# ----------------------------------------------------------------------------
# Entry point
# ----------------------------------------------------------------------------
def kernel(feats, pos, filt, dense_w, dense_b, src_idx, qry_idx):
    from concourse.bass_utils import run_bass_kernel_spmd

    feats = np.ascontiguousarray(np.asarray(feats, np.float32))
    pos = np.ascontiguousarray(np.asarray(pos, np.float32))
    filt = np.asarray(filt, np.float32)
    dense_w = np.asarray(dense_w, np.float32)
    dense_b = np.asarray(dense_b, np.float32)
    src_idx = np.asarray(src_idx).astype(np.int64)
    qry_idx = np.asarray(qry_idx).astype(np.int64)

    plans, bstart, bsz = _plan(qry_idx)
    NCH = max(len(p['chunks']) for p in plans)
    NCHP = ((NCH + 15) // 16) * 16
    NQ = NCHP * 16

    # filter regroup: G2[ax*32+c, t*64+o] = filt[az, ay, ax, c, o], t = az*4+ay
    G2 = np.zeros((128, 16 * 64), np.float32)
    for az in range(4):
        for ay in range(4):
            t = az * 4 + ay
            for ax in range(4):
                G2[ax * 32:(ax + 1) * 32, t * 64:(t + 1) * 64] = filt[az, ay, ax]

    in_maps = []
    for c, p in enumerate(plans):
        possrc, posqry, fsrc, qlocf = _pack_core(p, bstart, pos, feats,
                                                 qry_idx, src_idx, NCHP)
        ftT = np.zeros((CIN, NQ), np.float32)
        ftT[:, 0:p['nq']] = feats[p['q0']:p['q0'] + p['nq']].T
        in_maps.append({
            "possrc": possrc, "posqry": posqry, "fsrc": fsrc, "qlocf": qlocf,
            "g2": G2, "featsT": ftT, "denw": dense_w,
            "denb": dense_b.reshape(COUT, 1).astype(np.float32),
        })

    key = (NCHP, NQ)
    if key not in _COMPILED:
        _COMPILED[key] = _build_bass(NCHP, NQ)
    nc = _COMPILED[key]

    res = run_bass_kernel_spmd(nc, in_maps, core_ids=list(range(NCORES)))

    ans_conv = np.zeros((N, COUT), np.float32)
    ans_dense = np.zeros((N, COUT), np.float32)
    for c, p in enumerate(plans):
        outT = res.results[c]["outconvT"]
        for ci, (bA, bB) in enumerate(p['chunks']):
            for half, b in enumerate((bA, bB)):
                if b is None:
                    continue
                cols = ci * 16 + half * 8
                ans_conv[8 * b:8 * b + 8] = outT[:, cols:cols + 8].T
        dT = res.results[c]["outdenseT"]
        ans_dense[p['q0']:p['q0'] + p['nq']] = dT[:, 0:p['nq']].T
    return ans_conv, ans_dense



# revision 7
# speedup vs baseline: 1.7664x; 1.7664x over previous
"""Trainium2 Bass kernel for nn_ContinuousConvolutionBlock (gnn_message_passing).

Strategy (per sharding hint: partition points across 8 cores; each core owns its
queries' scatter-reduce and tap-GEMM; filter + dense weights replicated):

Host side (index plumbing / input marshalling only - zero FLOPs):
  - qry_idx is sorted; queries are grouped into 8-query blocks, blocks paired
    into 128-edge-slot "chunks" (two-pointer bin packing, ~3% padding).
  - Per-core per-slot payload arrays (pos[src], pos[qry], feats[src] in bf16,
    and the expanded query one-hot qexp[slot, chunk, hq, t] in bf16) are
    marshalled on host and DMA'd in dense layouts.  qexp is pure indexing
    (0/1 one-hot replicated over the 16 tap-pairs) - uploading it lets the
    DVE build the L matmul operand at 2x packed-bf16 rate.

Device side (all FLOP-bearing compute):
  - Geometry: ball->cube volume-preserving map on unscaled deltas (the map is
    homogeneous; the 2/EXTENT scale folds into the corner transform), with
    x/y lanes processed as [*, 2] pairs and reciprocal_approx_fast.
  - Corner weights via the tent identity  w4[ax] = relu(1 - |g - ax|)
    (equivalent to the (1-f, f) one-hot pair incl. boundary clipping): one
    DVE op for d4 = 37.5*m + 1.5 - ax over all 3 axes, two ACT ops for
    abs + relu -> w4 [128, W, 3, 4] bf16.
  - Scatter-reduce as factored matmul per chunk (bf16): R[slot,(ax,c)] =
    w4x (x) feats, L[slot,(hq,t)] = qexp (.) (w4z (x) w4y  broadcast);
    PE computes A^T[(ax,c),(hq,t)] = R^T @ L per chunk into PSUM.
    L/R are built in one fused DVE op per 16-chunk group; L runs in the
    2x_1P packed-bf16 mode (all operands innermost step-1).
  - PSUM->SBUF copies of A^T run on the scalar engine (ACT), overlapping the
    DVE operand builds for the next group.
  - Tap-GEMM: out^T += G_t^T @ A^T-slices over the 16 tap-pairs t (f32r).
  - Dense branch: out_dense^T = dense_w^T @ feats^T (bf16) + bias via ACT.
  Outputs are produced transposed ([64, nq]); host transposes/reorders back.
"""
import sys
import os
sys.path.insert(0, '/opt/trn_rl_repo')
import numpy as np
from ml_dtypes import bfloat16

N = 30000
CIN = 32
COUT = 64
KS = 4
EXTENT = 0.08
NCORES = 8
NBLK = N // 8  # 3750 eight-query blocks

_COMPILED = {}


# ----------------------------------------------------------------------------
# Host planning
# ----------------------------------------------------------------------------
def _plan(qry_idx):
    deg = np.bincount(qry_idx, minlength=N)
    bsz = deg.reshape(NBLK, 8).sum(1)
    bstart = np.concatenate([[0], np.cumsum(bsz)]).astype(np.int64)
    per = [NBLK // NCORES + (1 if c < NBLK % NCORES else 0) for c in range(NCORES)]
    b0 = np.concatenate([[0], np.cumsum(per)]).astype(np.int64)
    plans = []
    for c in range(NCORES):
        blocks = list(range(b0[c], b0[c + 1]))
        asc = sorted(blocks, key=lambda b: bsz[b])
        chunks = []
        lo, hi = 0, len(asc) - 1
        while lo <= hi:
            if lo == hi:
                chunks.append((asc[hi], None)); break
            if bsz[asc[hi]] + bsz[asc[lo]] <= 128:
                chunks.append((asc[hi], asc[lo])); hi -= 1; lo += 1
            else:
                chunks.append((asc[hi], None)); hi -= 1
        plans.append(dict(blocks=blocks, chunks=chunks, q0=int(8 * b0[c]),
                          nq=int(8 * (b0[c + 1] - b0[c]))))
    return plans, bstart, bsz


def _pack_core(plan_c, bstart, pos, feats, qry_idx, src_idx, NCHP):
    """Build per-slot payload arrays."""
    possrc = np.zeros((128, NCHP, 3), np.float32)
    posqry = np.zeros((128, NCHP, 3), np.float32)
    fsrc = np.zeros((128, NCHP, CIN), np.float32)
    qloc = np.full((128, NCHP), -1, np.int32)
    for ci, (bA, bB) in enumerate(plan_c['chunks']):
        s = 0
        for half, b in enumerate((bA, bB)):
            if b is None:
                continue
            e0, e1 = int(bstart[b]), int(bstart[b + 1])
            n = e1 - e0
            sl = slice(s, s + n)
            possrc[sl, ci, :] = pos[src_idx[e0:e1]]
            posqry[sl, ci, :] = pos[qry_idx[e0:e1]]
            fsrc[sl, ci, :] = feats[src_idx[e0:e1]]
            qloc[sl, ci] = (qry_idx[e0:e1] - 8 * b) + 8 * half
            s += n
    # expanded one-hot: qexp[s, w, hq, t] = (qloc[s,w] == hq), any t
    oh = (qloc[:, :, None] == np.arange(16, dtype=np.int32)[None, None, :])
    qexp = np.broadcast_to(oh[:, :, :, None], (128, NCHP, 16, 16))
    qexp = np.ascontiguousarray(qexp).astype(bfloat16).reshape(128, NCHP * 256)
    return possrc, posqry, fsrc.astype(bfloat16), qexp


def _prepare(feats, pos, filt, dense_w, dense_b, src_idx, qry_idx):
    """Plan + marshal all per-core input maps. Returns (NCHP, NQ, plans, in_maps)."""
    feats = np.ascontiguousarray(np.asarray(feats, np.float32))
    pos = np.ascontiguousarray(np.asarray(pos, np.float32))
    filt = np.asarray(filt, np.float32)
    dense_w = np.asarray(dense_w, np.float32)
    dense_b = np.asarray(dense_b, np.float32)
    src_idx = np.asarray(src_idx).astype(np.int64)
    qry_idx = np.asarray(qry_idx).astype(np.int64)

    plans, bstart, bsz = _plan(qry_idx)
    NCH = max(len(p['chunks']) for p in plans)
    NCHP = ((NCH + 15) // 16) * 16
    NQ = NCHP * 16

    # filter regroup: G2[ax*32+c, t*64+o] = filt[az, ay, ax, c, o], t = az*4+ay
    G2 = np.zeros((128, 16 * 64), np.float32)
    for az in range(4):
        for ay in range(4):
            t = az * 4 + ay
            for ax in range(4):
                G2[ax * 32:(ax + 1) * 32, t * 64:(t + 1) * 64] = filt[az, ay, ax]

    dwb = dense_w.astype(bfloat16)
    dbb = dense_b.reshape(COUT, 1).astype(np.float32)
    in_maps = []
    for c, p in enumerate(plans):
        possrc, posqry, fsrc, qexp = _pack_core(p, bstart, pos, feats,
                                                qry_idx, src_idx, NCHP)
        ftT = np.zeros((CIN, NQ), bfloat16)
        ftT[:, 0:p['nq']] = feats[p['q0']:p['q0'] + p['nq']].T.astype(bfloat16)
        in_maps.append({
            "possrc": possrc, "posqry": posqry, "fsrc": fsrc, "qexp": qexp,
            "g2": G2, "featsT": ftT, "denw": dwb, "denb": dbb,
        })
    return NCHP, NQ, plans, in_maps


# ----------------------------------------------------------------------------
# Device kernel
# ----------------------------------------------------------------------------
def _build_bass(NCHP, NQ):
    import concourse.bass as bass
    import concourse.tile as tile
    from concourse import bacc, mybir
    from concourse.bass import AP

    f32 = mybir.dt.float32
    f32r = mybir.dt.float32r
    bf16 = mybir.dt.bfloat16
    i32 = mybir.dt.int32
    ALU = mybir.AluOpType
    ACT = mybir.ActivationFunctionType
    W = NCHP
    NGRP = W // 16
    SCL = float(2.0 / EXTENT) * 1.5  # tent scale: g = SCL*m + 1.5
    F4PI = float(4.0 / np.pi)

    nc = bacc.Bacc("TRN2", target_bir_lowering=False, debug=False)

    possrc = nc.dram_tensor("possrc", (128, W, 3), f32, kind="ExternalInput")
    posqry = nc.dram_tensor("posqry", (128, W, 3), f32, kind="ExternalInput")
    fsrc = nc.dram_tensor("fsrc", (128, W, CIN), bf16, kind="ExternalInput")
    qexp = nc.dram_tensor("qexp", (128, W * 256), bf16, kind="ExternalInput")
    g2 = nc.dram_tensor("g2", (128, 16 * 64), f32, kind="ExternalInput")
    featsT = nc.dram_tensor("featsT", (CIN, NQ), bf16, kind="ExternalInput")
    denw = nc.dram_tensor("denw", (CIN, COUT), bf16, kind="ExternalInput")
    denb = nc.dram_tensor("denb", (COUT, 1), f32, kind="ExternalInput")

    outconvT = nc.dram_tensor("outconvT", (COUT, NQ), f32, kind="ExternalOutput")
    outdenseT = nc.dram_tensor("outdenseT", (COUT, NQ), f32, kind="ExternalOutput")

    with tile.TileContext(nc) as tc:
        with tc.tile_pool(name="inp", bufs=1) as inp, \
             tc.tile_pool(name="geo", bufs=1) as geo, \
             tc.tile_pool(name="tmp", bufs=1) as tmp, \
             tc.tile_pool(name="qex", bufs=4) as qex, \
             tc.tile_pool(name="lp", bufs=3) as lp, \
             tc.tile_pool(name="rp", bufs=3) as rp, \
             tc.tile_pool(name="atp", bufs=2) as atp, \
             tc.tile_pool(name="outp", bufs=4) as outp, \
             tc.tile_pool(name="ps1", bufs=4, space="PSUM") as ps1, \
             tc.tile_pool(name="ps2", bufs=2, space="PSUM") as ps2:

            # ---------------- input DMAs ----------------
            t_ps = inp.tile([128, W, 3], f32)
            t_pq = inp.tile([128, W, 3], f32)
            t_f = inp.tile([128, W, CIN], bf16)
            t_g2 = inp.tile([128, 16 * 64], f32)
            t_ftT = inp.tile([CIN, NQ], bf16)
            t_dw = inp.tile([CIN, COUT], bf16)
            t_db = inp.tile([COUT, 1], f32)
            nc.sync.dma_start(t_ps[:], possrc[:])
            nc.sync.dma_start(t_pq[:], posqry[:])
            nc.sync.dma_start(t_f[:], fsrc[:])
            nc.sync.dma_start(t_g2[:], g2[:])
            nc.sync.dma_start(t_ftT[:], featsT[:])
            nc.sync.dma_start(t_dw[:], denw[:])
            nc.sync.dma_start(t_db[:], denb[:])

            t_g2r = inp.tile([128, 16 * 64], f32r)
            nc.scalar.copy(t_g2r[:], t_g2[:])

            # ---------------- dense branch (overlaps geometry) ----------------
            for s_ in range((NQ + 511) // 512):
                j0 = s_ * 512
                j1 = min(NQ, j0 + 512)
                w_ = j1 - j0
                pd = ps2.tile([COUT, 512], f32, space="PSUM", tag="den")
                nc.tensor.matmul(out=pd[:, 0:w_], lhsT=t_dw[:],
                                 rhs=t_ftT[:, j0:j1], start=True, stop=True)
                odt = outp.tile([COUT, 512], f32, tag="odst")
                nc.scalar.activation(odt[:, 0:w_], pd[:, 0:w_], ACT.Identity,
                                     bias=t_db[:, 0:1], scale=1.0)
                nc.sync.dma_start(outdenseT[:, j0:j1], odt[:, 0:w_])

            # iota constant: io12s[axis*4 + ax] = ax - 1.5
            io12i = tmp.tile([128, 12], i32)
            nc.gpsimd.iota(io12i[:], pattern=[[0, 3], [1, 4]], base=0,
                           channel_multiplier=0)
            io12f = tmp.tile([128, 12], f32)
            nc.vector.tensor_copy(io12f[:], io12i[:])
            io12s = geo.tile([128, 12], f32)
            nc.vector.tensor_scalar(io12s[:], io12f[:], -1.5, None, op0=ALU.add)

            # ---------------- geometry ----------------
            _tn = [0]
            _free_tags = []
            _tag_of = {}
            _seq = [0]

            def T(shape=(128, W), dt_=f32):
                if _free_tags:
                    tg = _free_tags.pop()
                else:
                    _tn[0] += 1
                    tg = f"t{_tn[0]}"
                _seq[0] += 1
                t = tmp.tile(list(shape), dt_, name=f"{tg}_u{_seq[0]}", tag=tg)
                _tag_of[id(t)] = tg
                return t

            def F(*ts):
                for t in ts:
                    _free_tags.append(_tag_of.pop(id(t)))

            TT = nc.vector.tensor_tensor
            TS = nc.vector.tensor_scalar
            STT = nc.vector.scalar_tensor_tensor
            ACTV = nc.scalar.activation
            RCP = nc.vector.reciprocal_approx_fast

            def bc(t, n, stride=1):
                """broadcast [128, W] tile over trailing n"""
                return AP(t.tensor, t[:].offset, [t[:].ap[0], [stride, W], [0, n]])

            # d3 = ps - pq (unscaled; map is homogeneous, scale folds into tent)
            d3 = T((128, W, 3))
            TT(out=d3[:], in0=t_ps[:], in1=t_pq[:], op=ALU.subtract)
            x, y, z = d3[:, :, 0], d3[:, :, 1], d3[:, :, 2]
            sq3 = T((128, W, 3))
            ACTV(sq3[:], d3[:], ACT.Square)
            xy2 = T()
            TT(out=xy2[:], in0=sq3[:, :, 0], in1=sq3[:, :, 1], op=ALU.add)
            sq = T()
            TT(out=sq[:], in0=xy2[:], in1=sq3[:, :, 2], op=ALU.add)
            norm = T()
            ACTV(norm[:], sq[:], ACT.Sqrt)
            F(sq)
            nxy = T()
            ACTV(nxy[:], xy2[:], ACT.Sqrt)
            pole = T()
            STT(out=pole[:], in0=sq3[:, :, 2], scalar=1.25, in1=xy2[:],
                op0=ALU.mult, op1=ALU.is_gt)
            F(xy2, sq3)

            azn = T()
            ACTV(azn[:], z, ACT.Abs)
            den1 = T()
            STT(out=den1[:], in0=azn[:], scalar=1e-13, in1=norm[:],
                op0=ALU.add, op1=ALU.add)
            F(azn)
            rd1 = T()
            RCP(rd1[:], den1[:])
            F(den1)
            t1a = T()
            STT(out=t1a[:], in0=norm[:], scalar=3.0, in1=rd1[:],
                op0=ALU.mult, op1=ALU.mult)
            F(rd1)
            s1 = T()
            ACTV(s1[:], t1a[:], ACT.Sqrt)
            F(t1a)
            den2 = T()
            TS(den2[:], nxy[:], 1e-13, None, op0=ALU.add)
            F(nxy)
            rd2 = T()
            RCP(rd2[:], den2[:])
            F(den2)
            s2 = T()
            TT(out=s2[:], in0=norm[:], in1=rd2[:], op=ALU.mult)
            F(rd2)
            dd = T()
            TT(out=dd[:], in0=s1[:], in1=s2[:], op=ALU.subtract)
            F(s1)
            pw = T()
            TT(out=pw[:], in0=pole[:], in1=dd[:], op=ALU.mult)
            F(dd)
            wq = T()
            TT(out=wq[:], in0=s2[:], in1=pw[:], op=ALU.add)
            F(s2, pw)

            # m3 holds SCL * (cube coords): the tent scale is baked in here so
            # the d4 op below can be a plain TT (verifier caps TS-class ops
            # at 2 free dims).
            m3 = geo.tile([128, W, 3], f32)
            cyl2 = T((128, W, 2))
            TT(out=cyl2[:], in0=d3[:, :, 0:2], in1=bc(wq, 2), op=ALU.mult)
            F(wq)
            # z branch: SCL*zc = 1.5*SCL*z + pole*(sign(z)*SCL*norm - 1.5*SCL*z)
            sgz = T()
            ACTV(sgz[:], z, ACT.Sign)
            zcp = T()
            STT(out=zcp[:], in0=sgz[:], scalar=SCL, in1=norm[:],
                op0=ALU.mult, op1=ALU.mult)
            F(sgz, norm)
            u = T()
            STT(out=u[:], in0=z, scalar=-1.5 * SCL, in1=zcp[:],
                op0=ALU.mult, op1=ALU.add)
            F(zcp)
            pu = T()
            TT(out=pu[:], in0=pole[:], in1=u[:], op=ALU.mult)
            F(u, pole)
            STT(out=m3[:, :, 2], in0=z, scalar=1.5 * SCL, in1=pu[:],
                op0=ALU.mult, op1=ALU.add)
            F(pu, d3)

            # cylinder -> cube (x/y as [*, 2] pairs)
            sqc = T((128, W, 2))
            ACTV(sqc[:], cyl2[:], ACT.Square)
            sqxy = T()
            TT(out=sqxy[:], in0=sqc[:, :, 0], in1=sqc[:, :, 1], op=ALU.add)
            F(sqc)
            nrm = T()
            ACTV(nrm[:], sqxy[:], ACT.Sqrt)
            F(sqxy)
            acl = T((128, W, 2))
            ACTV(acl[:], cyl2[:], ACT.Abs)
            abr = T()
            TT(out=abr[:], in0=acl[:, :, 1], in1=acl[:, :, 0], op=ALU.is_le)
            m2 = T((128, W, 2))
            TS(m2[:], acl[:], 1e-12, None, op0=ALU.is_lt)
            F(acl)
            sf2 = T((128, W, 2))
            TT(out=sf2[:], in0=cyl2[:], in1=m2[:], op=ALU.add)
            F(m2)
            inv2 = T((128, W, 2))
            RCP(inv2[:], sf2[:])
            F(sf2)
            rat2 = T((128, W, 2))
            TT(out=rat2[:], in0=cyl2[:],
               in1=AP(inv2.tensor, inv2[:].offset + 1,
                      [inv2[:].ap[0], [2, W], [-1, 2]]),
               op=ALU.mult)
            F(inv2)
            at2v = T((128, W, 2))
            ACTV(at2v[:], rat2[:], ACT.Arctan)
            F(rat2)
            sg2 = T((128, W, 2))
            ACTV(sg2[:], cyl2[:], ACT.Sign)
            F(cyl2)
            t4 = T((128, W, 4))
            # t4[0:2] = (tmpa, tmpb) = sign(xc,yc) * nrm * SCL
            STT(out=t4[:, :, 0:2], in0=sg2[:], scalar=SCL, in1=bc(nrm, 2),
                op0=ALU.mult, op1=ALU.mult)
            F(sg2, nrm)
            # t4[2:4] = (xoe, yoe) = F4PI * arctan * (tmpb, tmpa)
            STT(out=t4[:, :, 2:4], in0=at2v[:], scalar=F4PI,
                in1=AP(t4.tensor, t4[:].offset + 1,
                       [t4[:].ap[0], [4, W], [-1, 2]]),
                op0=ALU.mult, op1=ALU.mult)
            F(at2v)
            # del2 = (tmpa - xoe, yoe - tmpb)
            del2 = T((128, W, 2))
            TT(out=del2[:],
               in0=AP(t4.tensor, t4[:].offset + 0, [t4[:].ap[0], [4, W], [3, 2]]),
               in1=AP(t4.tensor, t4[:].offset + 2, [t4[:].ap[0], [4, W], [-1, 2]]),
               op=ALU.subtract)
            ad2 = T((128, W, 2))
            TT(out=ad2[:], in0=del2[:], in1=bc(abr, 2), op=ALU.mult)
            F(del2, abr)
            # m3[:, :, 0:2] = (xoe, tmpb) + abr*delta
            TT(out=m3[:, :, 0:2],
               in0=AP(t4.tensor, t4[:].offset + 2, [t4[:].ap[0], [4, W], [-1, 2]]),
               in1=ad2[:], op=ALU.add)
            F(ad2, t4)

            # ---------------- tent corner weights ----------------
            # d4[s, w, axis, ax] = SCL*m + 1.5 - ax ; w4 = relu(1 - |d4|)
            d4 = T((128, W, 3, 4))
            TT(out=d4[:],
               in0=AP(m3.tensor, m3[:].offset,
                      [m3[:].ap[0], [3, W], [1, 3], [0, 4]]),
               in1=AP(io12s.tensor, io12s[:].offset,
                      [io12s[:].ap[0], [0, W], [4, 3], [1, 4]]),
               op=ALU.subtract)
            a4 = T((128, W, 3, 4))
            ACTV(a4[:], d4[:], ACT.Abs)
            F(d4)
            w4 = geo.tile([128, W, 3, 4], bf16)
            ACTV(w4[:], a4[:], ACT.Relu, bias=1.0, scale=-1.0)
            F(a4)

            # zy[s, w, az, ay] = w4z (x) w4y   (bf16)
            zy = geo.tile([128, W, 16], bf16)
            TT(out=AP(zy.tensor, zy[:].offset,
                      [zy[:].ap[0], [16, W], [4, 4], [1, 4]]),
               in0=AP(w4.tensor, w4[:].offset + 8,
                      [w4[:].ap[0], [12, W], [1, 4], [0, 4]]),
               in1=AP(w4.tensor, w4[:].offset + 4,
                      [w4[:].ap[0], [12, W], [0, 4], [1, 4]]),
               op=ALU.mult)

            # ---------------- per-group: build L/R, stage-1, tap-GEMM --------
            for g in range(NGRP):
                c0 = g * 16
                tq = qex.tile([128, 16 * 256], bf16, tag="qex")
                nc.sync.dma_start(tq[:], qexp[:, g * 4096:(g + 1) * 4096])

                # L[s, ch, hq, t] = qexp * zy (packed bf16 2x mode)
                L = lp.tile([128, 16 * 256], bf16, tag="L")
                TT(out=AP(L.tensor, L[:].offset,
                          [L[:].ap[0], [256, 16], [16, 16], [1, 16]]),
                   in0=AP(tq.tensor, tq[:].offset,
                          [tq[:].ap[0], [256, 16], [16, 16], [1, 16]]),
                   in1=AP(zy.tensor, zy[:].offset + c0 * 16,
                          [zy[:].ap[0], [16, 16], [0, 16], [1, 16]]),
                   op=ALU.mult)

                # R[s, ch, ax, c] = w4x (x) feats (bf16)
                R = rp.tile([128, 16 * 128], bf16, tag="R")
                TT(out=AP(R.tensor, R[:].offset,
                          [R[:].ap[0], [128, 16], [32, 4], [1, 32]]),
                   in0=AP(w4.tensor, w4[:].offset + c0 * 12,
                          [w4[:].ap[0], [12, 16], [1, 4], [0, 32]]),
                   in1=AP(t_f.tensor, t_f[:].offset + c0 * 32,
                          [t_f[:].ap[0], [32, 16], [0, 4], [1, 32]]),
                   op=ALU.mult)

                at_st = atp.tile([128, 16 * 256], f32r, tag="at")
                for cl in range(0, 16, 2):
                    ps_t = ps1.tile([128, 512], f32, space="PSUM", tag="s1")
                    for par in range(2):
                        ci = cl + par
                        nc.tensor.matmul(
                            out=ps_t[:, par * 256:(par + 1) * 256],
                            lhsT=R[:, ci * 128:(ci + 1) * 128],
                            rhs=L[:, ci * 256:(ci + 1) * 256],
                            start=True, stop=True)
                    # A^T copies ride the scalar engine; DVE builds next L/R
                    nc.scalar.copy(at_st[:, cl * 256:(cl + 2) * 256], ps_t[:])

                po = ps2.tile([COUT, 256], f32, space="PSUM", tag="tap")
                for t in range(16):
                    rhs = AP(at_st.tensor, at_st[:].offset + t,
                             [at_st[:].ap[0], [256, 16], [128, 2], [16, 8]])
                    nc.tensor.matmul(
                        out=po[:],
                        lhsT=t_g2r[:, t * 64:(t + 1) * 64],
                        rhs=rhs,
                        start=(t == 0), stop=(t == 15))
                ost = outp.tile([COUT, 256], f32, tag="ocst")
                if g % 2 == 0:
                    nc.vector.tensor_copy(ost[:], po[:])
                else:
                    nc.scalar.copy(ost[:], po[:])
                nc.sync.dma_start(outconvT[:, g * 256:(g + 1) * 256], ost[:])

    nc.compile()
    return nc


# ----------------------------------------------------------------------------
# Entry point
# ----------------------------------------------------------------------------
def kernel(feats, pos, filt, dense_w, dense_b, src_idx, qry_idx):
    from concourse.bass_utils import run_bass_kernel_spmd

    NCHP, NQ, plans, in_maps = _prepare(feats, pos, filt, dense_w, dense_b,
                                        src_idx, qry_idx)
    key = (NCHP, NQ)
    if key not in _COMPILED:
        _COMPILED[key] = _build_bass(NCHP, NQ)
    nc = _COMPILED[key]

    res = run_bass_kernel_spmd(nc, in_maps, core_ids=list(range(NCORES)))

    ans_conv = np.zeros((N, COUT), np.float32)
    ans_dense = np.zeros((N, COUT), np.float32)
    for c, p in enumerate(plans):
        outT = res.results[c]["outconvT"]
        for ci, (bA, bB) in enumerate(p['chunks']):
            for half, b in enumerate((bA, bB)):
                if b is None:
                    continue
                cols = ci * 16 + half * 8
                ans_conv[8 * b:8 * b + 8] = outT[:, cols:cols + 8].T
        dT = res.results[c]["outdenseT"]
        ans_dense[p['q0']:p['q0'] + p['nq']] = dT[:, 0:p['nq']].T
    return ans_conv, ans_dense


# revision 9
# speedup vs baseline: 2.0473x; 1.1590x over previous
"""Trainium2 Bass kernel for nn_ContinuousConvolutionBlock (gnn_message_passing).

Strategy (per sharding hint: partition points across 8 cores; each core owns its
queries' scatter-reduce and tap-GEMM; filter + dense weights replicated):

Host side (index plumbing / input marshalling only - zero FLOPs):
  - qry_idx is sorted; queries are grouped into 8-query blocks, blocks paired
    into 128-edge-slot "chunks" (two-pointer bin packing, ~3% padding).
  - Per-core per-slot payload arrays (pos[src], pos[qry], feats[src] in bf16,
    and the expanded query one-hot qexp[slot, chunk, hq, t] in bf16) are
    marshalled on host and DMA'd in dense layouts.  qexp is pure indexing
    (0/1 one-hot replicated over the 16 tap-pairs) - uploading it lets the
    DVE build the L matmul operand at 2x packed-bf16 rate.

Device side (all FLOP-bearing compute):
  - Geometry: ball->cube volume-preserving map on unscaled deltas (the map is
    homogeneous; the 2/EXTENT scale folds into the corner transform), with
    x/y lanes processed as [*, 2] pairs and reciprocal_approx_fast.
  - Corner weights via the tent identity  w4[ax] = relu(1 - |g - ax|)
    (equivalent to the (1-f, f) one-hot pair incl. boundary clipping): one
    DVE op for d4 = 37.5*m + 1.5 - ax over all 3 axes, two ACT ops for
    abs + relu -> w4 [128, W, 3, 4] bf16.
  - Scatter-reduce as factored matmul per chunk (bf16): R[slot,(ax,c)] =
    w4x (x) feats, L[slot,(hq,t)] = qexp (.) (w4z (x) w4y  broadcast);
    PE computes A^T[(ax,c),(hq,t)] = R^T @ L per chunk into PSUM.
    L/R are built in one fused DVE op per 16-chunk group; L runs in the
    2x_1P packed-bf16 mode (all operands innermost step-1).
  - PSUM->SBUF copies of A^T run on the scalar engine (ACT), overlapping the
    DVE operand builds for the next group.
  - Tap-GEMM: out^T += G_t^T @ A^T-slices over the 16 tap-pairs t (f32r).
  - Dense branch: out_dense^T = dense_w^T @ feats^T (bf16) + bias via ACT.
  Outputs are produced transposed ([64, nq]); host transposes/reorders back.
"""
import sys
import os
sys.path.insert(0, '/opt/trn_rl_repo')
import numpy as np
from ml_dtypes import bfloat16

N = 30000
CIN = 32
COUT = 64
KS = 4
EXTENT = 0.08
NCORES = 8
NBLK = N // 8  # 3750 eight-query blocks

_COMPILED = {}


# ----------------------------------------------------------------------------
# Host planning
# ----------------------------------------------------------------------------
def _plan(qry_idx):
    deg = np.bincount(qry_idx, minlength=N)
    bsz = deg.reshape(NBLK, 8).sum(1)
    bstart = np.concatenate([[0], np.cumsum(bsz)]).astype(np.int64)
    per = [NBLK // NCORES + (1 if c < NBLK % NCORES else 0) for c in range(NCORES)]
    b0 = np.concatenate([[0], np.cumsum(per)]).astype(np.int64)
    plans = []
    for c in range(NCORES):
        blocks = list(range(b0[c], b0[c + 1]))
        asc = sorted(blocks, key=lambda b: bsz[b])
        chunks = []
        lo, hi = 0, len(asc) - 1
        while lo <= hi:
            if lo == hi:
                chunks.append((asc[hi], None)); break
            if bsz[asc[hi]] + bsz[asc[lo]] <= 128:
                chunks.append((asc[hi], asc[lo])); hi -= 1; lo += 1
            else:
                chunks.append((asc[hi], None)); hi -= 1
        plans.append(dict(blocks=blocks, chunks=chunks, q0=int(8 * b0[c]),
                          nq=int(8 * (b0[c + 1] - b0[c]))))
    return plans, bstart, bsz


def _pack_core(plan_c, bstart, pos, feats, qry_idx, src_idx, NCHP):
    """Build per-slot payload arrays."""
    possrc = np.zeros((128, NCHP, 3), np.float32)
    posqry = np.zeros((128, NCHP, 3), np.float32)
    fsrc = np.zeros((128, NCHP, CIN), np.float32)
    qloc = np.full((128, NCHP), -1, np.int32)
    for ci, (bA, bB) in enumerate(plan_c['chunks']):
        s = 0
        for half, b in enumerate((bA, bB)):
            if b is None:
                continue
            e0, e1 = int(bstart[b]), int(bstart[b + 1])
            n = e1 - e0
            sl = slice(s, s + n)
            possrc[sl, ci, :] = pos[src_idx[e0:e1]]
            posqry[sl, ci, :] = pos[qry_idx[e0:e1]]
            fsrc[sl, ci, :] = feats[src_idx[e0:e1]]
            qloc[sl, ci] = (qry_idx[e0:e1] - 8 * b) + 8 * half
            s += n
    # expanded one-hot: qexp[s, w, hq, t] = (qloc[s,w] == hq), any t
    oh = (qloc[:, :, None] == np.arange(16, dtype=np.int32)[None, None, :])
    qexp = np.broadcast_to(oh[:, :, :, None], (128, NCHP, 16, 16))
    qexp = np.ascontiguousarray(qexp).astype(bfloat16).reshape(128, NCHP * 256)
    return possrc, posqry, fsrc.astype(bfloat16), qexp


def _prepare(feats, pos, filt, dense_w, dense_b, src_idx, qry_idx):
    """Plan + marshal all per-core input maps. Returns (NCHP, NQ, plans, in_maps)."""
    feats = np.ascontiguousarray(np.asarray(feats, np.float32))
    pos = np.ascontiguousarray(np.asarray(pos, np.float32))
    filt = np.asarray(filt, np.float32)
    dense_w = np.asarray(dense_w, np.float32)
    dense_b = np.asarray(dense_b, np.float32)
    src_idx = np.asarray(src_idx).astype(np.int64)
    qry_idx = np.asarray(qry_idx).astype(np.int64)

    plans, bstart, bsz = _plan(qry_idx)
    NCH = max(len(p['chunks']) for p in plans)
    NCHP = ((NCH + 15) // 16) * 16
    NQ = NCHP * 16

    # filter regroup: G2[ax*32+c, t*64+o] = filt[az, ay, ax, c, o], t = az*4+ay
    G2 = np.zeros((128, 16 * 64), np.float32)
    for az in range(4):
        for ay in range(4):
            t = az * 4 + ay
            for ax in range(4):
                G2[ax * 32:(ax + 1) * 32, t * 64:(t + 1) * 64] = filt[az, ay, ax]

    dwb = dense_w.astype(bfloat16)
    dbb = dense_b.reshape(COUT, 1).astype(np.float32)
    in_maps = []
    for c, p in enumerate(plans):
        possrc, posqry, fsrc, qexp = _pack_core(p, bstart, pos, feats,
                                                qry_idx, src_idx, NCHP)
        ftT = np.zeros((CIN, NQ), bfloat16)
        ftT[:, 0:p['nq']] = feats[p['q0']:p['q0'] + p['nq']].T.astype(bfloat16)
        in_maps.append({
            "possrc": possrc, "posqry": posqry, "fsrc": fsrc, "qexp": qexp,
            "g2": G2, "featsT": ftT, "denw": dwb, "denb": dbb,
        })
    return NCHP, NQ, plans, in_maps


# ----------------------------------------------------------------------------
# Device kernel
# ----------------------------------------------------------------------------
def _build_bass(NCHP, NQ):
    import concourse.bass as bass
    import concourse.tile as tile
    from concourse import bacc, mybir
    from concourse.bass import AP

    f32 = mybir.dt.float32
    f32r = mybir.dt.float32r
    bf16 = mybir.dt.bfloat16
    i32 = mybir.dt.int32
    ALU = mybir.AluOpType
    ACT = mybir.ActivationFunctionType
    W = NCHP
    NGRP = W // 16
    SCL = float(2.0 / EXTENT) * 1.5  # tent scale: g = SCL*m + 1.5
    F4PI = float(4.0 / np.pi)

    nc = bacc.Bacc("TRN2", target_bir_lowering=False, debug=False)

    possrc = nc.dram_tensor("possrc", (128, W, 3), f32, kind="ExternalInput")
    posqry = nc.dram_tensor("posqry", (128, W, 3), f32, kind="ExternalInput")
    fsrc = nc.dram_tensor("fsrc", (128, W, CIN), bf16, kind="ExternalInput")
    qexp = nc.dram_tensor("qexp", (128, W * 256), bf16, kind="ExternalInput")
    g2 = nc.dram_tensor("g2", (128, 16 * 64), f32, kind="ExternalInput")
    featsT = nc.dram_tensor("featsT", (CIN, NQ), bf16, kind="ExternalInput")
    denw = nc.dram_tensor("denw", (CIN, COUT), bf16, kind="ExternalInput")
    denb = nc.dram_tensor("denb", (COUT, 1), f32, kind="ExternalInput")

    outconvT = nc.dram_tensor("outconvT", (COUT, NQ), f32, kind="ExternalOutput")
    outdenseT = nc.dram_tensor("outdenseT", (COUT, NQ), f32, kind="ExternalOutput")

    with tile.TileContext(nc) as tc:
        with tc.tile_pool(name="inp", bufs=1) as inp, \
             tc.tile_pool(name="geo", bufs=1) as geo, \
             tc.tile_pool(name="tmp", bufs=1) as tmp, \
             tc.tile_pool(name="qex", bufs=4) as qex, \
             tc.tile_pool(name="lp", bufs=3) as lp, \
             tc.tile_pool(name="rp", bufs=3) as rp, \
             tc.tile_pool(name="atp", bufs=2) as atp, \
             tc.tile_pool(name="outp", bufs=4) as outp, \
             tc.tile_pool(name="ps1", bufs=3, space="PSUM") as ps1, \
             tc.tile_pool(name="ps2", bufs=1, space="PSUM") as ps2:

            # ---------------- input DMAs ----------------
            t_ps = inp.tile([128, W, 3], f32)
            t_pq = inp.tile([128, W, 3], f32)
            t_f = inp.tile([128, W, CIN], bf16)
            t_g2 = inp.tile([128, 16 * 64], f32)
            t_ftT = inp.tile([CIN, NQ], bf16)
            t_dw = inp.tile([CIN, COUT], bf16)
            t_db = inp.tile([COUT, 1], f32)
            nc.sync.dma_start(t_ps[:], possrc[:])
            nc.sync.dma_start(t_pq[:], posqry[:])
            nc.sync.dma_start(t_f[:], fsrc[:])
            nc.sync.dma_start(t_g2[:], g2[:])
            nc.sync.dma_start(t_ftT[:], featsT[:])
            nc.sync.dma_start(t_dw[:], denw[:])
            nc.sync.dma_start(t_db[:], denb[:])

            t_g2r = inp.tile([128, 16 * 64], f32r)
            nc.scalar.copy(t_g2r[:], t_g2[:])

            # ---------------- dense branch (overlaps geometry) ----------------
            for s_ in range((NQ + 511) // 512):
                j0 = s_ * 512
                j1 = min(NQ, j0 + 512)
                w_ = j1 - j0
                pd = ps2.tile([COUT, 512], f32, space="PSUM", tag="den")
                nc.tensor.matmul(out=pd[:, 0:w_], lhsT=t_dw[:],
                                 rhs=t_ftT[:, j0:j1], start=True, stop=True)
                odt = outp.tile([COUT, 512], f32, tag="odst")
                nc.scalar.activation(odt[:, 0:w_], pd[:, 0:w_], ACT.Identity,
                                     bias=t_db[:, 0:1], scale=1.0)
                nc.sync.dma_start(outdenseT[:, j0:j1], odt[:, 0:w_])

            # iota constant: io12s[axis*4 + ax] = ax - 1.5
            io12i = tmp.tile([128, 12], i32)
            nc.gpsimd.iota(io12i[:], pattern=[[0, 3], [1, 4]], base=0,
                           channel_multiplier=0)
            io12f = tmp.tile([128, 12], f32)
            nc.vector.tensor_copy(io12f[:], io12i[:])
            io12s = geo.tile([128, 12], f32)
            nc.vector.tensor_scalar(io12s[:], io12f[:], -1.5, None, op0=ALU.add)

            # ---------------- geometry ----------------
            _tn = [0]
            _free_tags = []
            _tag_of = {}
            _seq = [0]

            def T(shape=(128, W), dt_=f32):
                if _free_tags:
                    tg = _free_tags.pop()
                else:
                    _tn[0] += 1
                    tg = f"t{_tn[0]}"
                _seq[0] += 1
                t = tmp.tile(list(shape), dt_, name=f"{tg}_u{_seq[0]}", tag=tg)
                _tag_of[id(t)] = tg
                return t

            def F(*ts):
                for t in ts:
                    _free_tags.append(_tag_of.pop(id(t)))

            TT = nc.vector.tensor_tensor
            TS = nc.vector.tensor_scalar
            STT = nc.vector.scalar_tensor_tensor
            ACTV = nc.scalar.activation
            RCP = nc.vector.reciprocal_approx_fast

            def bc(t, n, stride=1):
                """broadcast [128, W] tile over trailing n"""
                return AP(t.tensor, t[:].offset, [t[:].ap[0], [stride, W], [0, n]])

            # d3 = ps - pq (unscaled; map is homogeneous, scale folds into tent)
            d3 = T((128, W, 3))
            TT(out=d3[:], in0=t_ps[:], in1=t_pq[:], op=ALU.subtract)
            x, y, z = d3[:, :, 0], d3[:, :, 1], d3[:, :, 2]
            sq3 = T((128, W, 3))
            ACTV(sq3[:], d3[:], ACT.Square)
            xy2 = T()
            TT(out=xy2[:], in0=sq3[:, :, 0], in1=sq3[:, :, 1], op=ALU.add)
            sq = T()
            TT(out=sq[:], in0=xy2[:], in1=sq3[:, :, 2], op=ALU.add)
            norm = T()
            ACTV(norm[:], sq[:], ACT.Sqrt)
            F(sq)
            nxy = T()
            ACTV(nxy[:], xy2[:], ACT.Sqrt)
            pole = T()
            STT(out=pole[:], in0=sq3[:, :, 2], scalar=1.25, in1=xy2[:],
                op0=ALU.mult, op1=ALU.is_gt)
            F(xy2, sq3)

            azn = T()
            ACTV(azn[:], z, ACT.Abs)
            den1 = T()
            STT(out=den1[:], in0=azn[:], scalar=1e-13, in1=norm[:],
                op0=ALU.add, op1=ALU.add)
            F(azn)
            rd1 = T()
            RCP(rd1[:], den1[:])
            F(den1)
            t1a = T()
            STT(out=t1a[:], in0=norm[:], scalar=3.0, in1=rd1[:],
                op0=ALU.mult, op1=ALU.mult)
            F(rd1)
            s1 = T()
            ACTV(s1[:], t1a[:], ACT.Sqrt)
            F(t1a)
            den2 = T()
            TS(den2[:], nxy[:], 1e-13, None, op0=ALU.add)
            F(nxy)
            rd2 = T()
            RCP(rd2[:], den2[:])
            F(den2)
            s2 = T()
            TT(out=s2[:], in0=norm[:], in1=rd2[:], op=ALU.mult)
            F(rd2)
            dd = T()
            TT(out=dd[:], in0=s1[:], in1=s2[:], op=ALU.subtract)
            F(s1)
            pw = T()
            TT(out=pw[:], in0=pole[:], in1=dd[:], op=ALU.mult)
            F(dd)
            wq = T()
            TT(out=wq[:], in0=s2[:], in1=pw[:], op=ALU.add)
            F(s2, pw)

            # m3 holds SCL * (cube coords): the tent scale is baked in here so
            # the d4 op below can be a plain TT (verifier caps TS-class ops
            # at 2 free dims).
            m3 = geo.tile([128, W, 3], f32)
            cyl2 = T((128, W, 2))
            TT(out=cyl2[:], in0=d3[:, :, 0:2], in1=bc(wq, 2), op=ALU.mult)
            F(wq)
            # z branch: SCL*zc = 1.5*SCL*z + pole*(sign(z)*SCL*norm - 1.5*SCL*z)
            sgz = T()
            ACTV(sgz[:], z, ACT.Sign)
            zcp = T()
            STT(out=zcp[:], in0=sgz[:], scalar=SCL, in1=norm[:],
                op0=ALU.mult, op1=ALU.mult)
            F(sgz, norm)
            u = T()
            STT(out=u[:], in0=z, scalar=-1.5 * SCL, in1=zcp[:],
                op0=ALU.mult, op1=ALU.add)
            F(zcp)
            pu = T()
            TT(out=pu[:], in0=pole[:], in1=u[:], op=ALU.mult)
            F(u, pole)
            STT(out=m3[:, :, 2], in0=z, scalar=1.5 * SCL, in1=pu[:],
                op0=ALU.mult, op1=ALU.add)
            F(pu, d3)

            # cylinder -> cube (x/y as [*, 2] pairs)
            sqc = T((128, W, 2))
            ACTV(sqc[:], cyl2[:], ACT.Square)
            sqxy = T()
            TT(out=sqxy[:], in0=sqc[:, :, 0], in1=sqc[:, :, 1], op=ALU.add)
            F(sqc)
            nrm = T()
            ACTV(nrm[:], sqxy[:], ACT.Sqrt)
            F(sqxy)
            acl = T((128, W, 2))
            ACTV(acl[:], cyl2[:], ACT.Abs)
            abr = T()
            TT(out=abr[:], in0=acl[:, :, 1], in1=acl[:, :, 0], op=ALU.is_le)
            m2 = T((128, W, 2))
            TS(m2[:], acl[:], 1e-12, None, op0=ALU.is_lt)
            F(acl)
            sf2 = T((128, W, 2))
            TT(out=sf2[:], in0=cyl2[:], in1=m2[:], op=ALU.add)
            F(m2)
            inv2 = T((128, W, 2))
            RCP(inv2[:], sf2[:])
            F(sf2)
            rat2 = T((128, W, 2))
            TT(out=rat2[:], in0=cyl2[:],
               in1=AP(inv2.tensor, inv2[:].offset + 1,
                      [inv2[:].ap[0], [2, W], [-1, 2]]),
               op=ALU.mult)
            F(inv2)
            at2v = T((128, W, 2))
            ACTV(at2v[:], rat2[:], ACT.Arctan)
            F(rat2)
            sg2 = T((128, W, 2))
            ACTV(sg2[:], cyl2[:], ACT.Sign)
            F(cyl2)
            t4 = T((128, W, 4))
            # t4[0:2] = (tmpa, tmpb) = sign(xc,yc) * nrm * SCL
            STT(out=t4[:, :, 0:2], in0=sg2[:], scalar=SCL, in1=bc(nrm, 2),
                op0=ALU.mult, op1=ALU.mult)
            F(sg2, nrm)
            # t4[2:4] = (xoe, yoe) = F4PI * arctan * (tmpb, tmpa)
            STT(out=t4[:, :, 2:4], in0=at2v[:], scalar=F4PI,
                in1=AP(t4.tensor, t4[:].offset + 1,
                       [t4[:].ap[0], [4, W], [-1, 2]]),
                op0=ALU.mult, op1=ALU.mult)
            F(at2v)
            # del2 = (tmpa - xoe, yoe - tmpb)
            del2 = T((128, W, 2))
            TT(out=del2[:],
               in0=AP(t4.tensor, t4[:].offset + 0, [t4[:].ap[0], [4, W], [3, 2]]),
               in1=AP(t4.tensor, t4[:].offset + 2, [t4[:].ap[0], [4, W], [-1, 2]]),
               op=ALU.subtract)
            ad2 = T((128, W, 2))
            TT(out=ad2[:], in0=del2[:], in1=bc(abr, 2), op=ALU.mult)
            F(del2, abr)
            # m3[:, :, 0:2] = (xoe, tmpb) + abr*delta
            TT(out=m3[:, :, 0:2],
               in0=AP(t4.tensor, t4[:].offset + 2, [t4[:].ap[0], [4, W], [-1, 2]]),
               in1=ad2[:], op=ALU.add)
            F(ad2, t4)

            # ---------------- tent corner weights ----------------
            # d4[s, w, axis, ax] = SCL*m + 1.5 - ax ; w4 = relu(1 - |d4|)
            d4 = T((128, W, 3, 4))
            TT(out=d4[:],
               in0=AP(m3.tensor, m3[:].offset,
                      [m3[:].ap[0], [3, W], [1, 3], [0, 4]]),
               in1=AP(io12s.tensor, io12s[:].offset,
                      [io12s[:].ap[0], [0, W], [4, 3], [1, 4]]),
               op=ALU.subtract)
            a4 = T((128, W, 3, 4))
            ACTV(a4[:], d4[:], ACT.Abs)
            F(d4)
            w4 = geo.tile([128, W, 3, 4], bf16)
            ACTV(w4[:], a4[:], ACT.Relu, bias=1.0, scale=-1.0)
            F(a4)

            # zy[s, w, az, ay] = w4z (x) w4y   (bf16)
            zy = geo.tile([128, W, 16], bf16)
            TT(out=AP(zy.tensor, zy[:].offset,
                      [zy[:].ap[0], [16, W], [4, 4], [1, 4]]),
               in0=AP(w4.tensor, w4[:].offset + 8,
                      [w4[:].ap[0], [12, W], [1, 4], [0, 4]]),
               in1=AP(w4.tensor, w4[:].offset + 4,
                      [w4[:].ap[0], [12, W], [0, 4], [1, 4]]),
               op=ALU.mult)

            # ---------------- per-group: build L/R, stage-1, tap-GEMM --------
            # Software-pipelined: tap-GEMM of group g-1 is emitted after
            # stage-1 of group g, so the PE stays fed while the scalar engine
            # drains PSUM->SBUF copies of the previous group.
            at_tiles = {}

            def tap_gemm(g):
                at_prev = at_tiles.pop(g)
                po = ps2.tile([COUT, 256], f32, space="PSUM", tag="tap")
                for t in range(16):
                    rhs = AP(at_prev.tensor, at_prev[:].offset + t,
                             [at_prev[:].ap[0], [256, 16], [128, 2], [16, 8]])
                    nc.tensor.matmul(
                        out=po[:],
                        lhsT=t_g2r[:, t * 64:(t + 1) * 64],
                        rhs=rhs,
                        start=(t == 0), stop=(t == 15))
                ost = outp.tile([COUT, 256], f32, tag="ocst")
                if g % 2 == 0:
                    nc.vector.tensor_copy(ost[:], po[:])
                else:
                    nc.scalar.copy(ost[:], po[:])
                nc.sync.dma_start(outconvT[:, g * 256:(g + 1) * 256], ost[:])

            for g in range(NGRP):
                c0 = g * 16
                tq = qex.tile([128, 16 * 256], bf16, tag="qex")
                nc.sync.dma_start(tq[:], qexp[:, g * 4096:(g + 1) * 4096])

                # L[s, ch, hq, t] = qexp * zy (packed bf16 2x mode)
                L = lp.tile([128, 16 * 256], bf16, tag="L")
                TT(out=AP(L.tensor, L[:].offset,
                          [L[:].ap[0], [256, 16], [16, 16], [1, 16]]),
                   in0=AP(tq.tensor, tq[:].offset,
                          [tq[:].ap[0], [256, 16], [16, 16], [1, 16]]),
                   in1=AP(zy.tensor, zy[:].offset + c0 * 16,
                          [zy[:].ap[0], [16, 16], [0, 16], [1, 16]]),
                   op=ALU.mult)

                # R[s, ch, ax, c] = w4x (x) feats (bf16)
                R = rp.tile([128, 16 * 128], bf16, tag="R")
                TT(out=AP(R.tensor, R[:].offset,
                          [R[:].ap[0], [128, 16], [32, 4], [1, 32]]),
                   in0=AP(w4.tensor, w4[:].offset + c0 * 12,
                          [w4[:].ap[0], [12, 16], [1, 4], [0, 32]]),
                   in1=AP(t_f.tensor, t_f[:].offset + c0 * 32,
                          [t_f[:].ap[0], [32, 16], [0, 4], [1, 32]]),
                   op=ALU.mult)

                at_st = atp.tile([128, 16 * 256], f32r, tag="at")
                at_tiles[g] = at_st
                for cl in range(0, 16, 4):
                    ps_t = ps1.tile([128, 1024], f32, space="PSUM", tag="s1")
                    for par in range(4):
                        ci = cl + par
                        nc.tensor.matmul(
                            out=ps_t[:, par * 256:(par + 1) * 256],
                            lhsT=R[:, ci * 128:(ci + 1) * 128],
                            rhs=L[:, ci * 256:(ci + 1) * 256],
                            start=True, stop=True)
                    # A^T copies ride the scalar engine; DVE builds next L/R
                    nc.scalar.copy(at_st[:, cl * 256:(cl + 4) * 256], ps_t[:])
                if g >= 1:
                    tap_gemm(g - 1)
            tap_gemm(NGRP - 1)

    nc.compile()
    return nc


# ----------------------------------------------------------------------------
# Entry point
# ----------------------------------------------------------------------------
def kernel(feats, pos, filt, dense_w, dense_b, src_idx, qry_idx):
    from concourse.bass_utils import run_bass_kernel_spmd

    NCHP, NQ, plans, in_maps = _prepare(feats, pos, filt, dense_w, dense_b,
                                        src_idx, qry_idx)
    key = (NCHP, NQ)
    if key not in _COMPILED:
        _COMPILED[key] = _build_bass(NCHP, NQ)
    nc = _COMPILED[key]

    res = run_bass_kernel_spmd(nc, in_maps, core_ids=list(range(NCORES)))

    ans_conv = np.zeros((N, COUT), np.float32)
    ans_dense = np.zeros((N, COUT), np.float32)
    for c, p in enumerate(plans):
        outT = res.results[c]["outconvT"]
        for ci, (bA, bB) in enumerate(p['chunks']):
            for half, b in enumerate((bA, bB)):
                if b is None:
                    continue
                cols = ci * 16 + half * 8
                ans_conv[8 * b:8 * b + 8] = outT[:, cols:cols + 8].T
        dT = res.results[c]["outdenseT"]
        ans_dense[p['q0']:p['q0'] + p['nq']] = dT[:, 0:p['nq']].T
    return ans_conv, ans_dense
